# revision 46
# baseline (speedup 1.0000x reference)
"""Trainium2 Bass kernel for nn_AttentionConv2D (two conv3x3+BN branches with
position-aware attention maps), SPMD over 8 NeuronCores. v3.

Sharding: core = batch_index * 2 + h_half. Each core computes both branches for
one batch element's 128-row horizontal slab (plus 1-row halo for the attention
3x3 conv). All cross-core data movement is done host-side; the device program
is identical on every core.

Device dataflow per core (f-row coords fr in [0,130), output rows fr in [1,129)):
  conv3x3 (both branches, 128 out ch) -> implicit GEMM, bf16 x/weights,
    9 taps x row-pair matmuls (N=512) accumulated in f32 PSUM, 4-row groups
  f = psum * scale_c + bias_c          (ScalarE evac -> bf16 FB tiles)
  att = A^T @ f                        (PE, M=2 col-tiled per quad of 4 groups
    via tile_position, so the 4 groups' matmuls overlap on the PE array)
  att evac                             (one wide VectorE copy per full quad;
    ScalarE per-group copies for the partial tail batches)
  att row-layout repack                (DMA into per-block att_rt tiles; whole
    4-row groups move as one multi-dim-AP single-packet DMA, branches split
    across the scalar/sync HWDGE queues; pos2 rows prestaged at startup)
  z2 = banded-matrix matmuls over rows (PE, K=2bw+2, both branches into one
    PSUM bank)
  map = sigmoid(z2)                    (single ScalarE sigmoid -> bf16 mapS)
  map broadcast to 128 partitions      (DMA to partitions 0/32/64/96 +
    VectorE stream_shuffle run in 1x mode only, so the APs are bitcast to f32
    to halve the streamed element count)
  out = f * map                        (VectorE bf16 in-place) ->
    gpsimd cast-DMA bf16->f32 to DRAM

Tail scheduling: blocks narrow toward the bottom ((97,14),(111,8),(119,8),
(127,2)) with staggered gating conv groups so each block's serial chain
overlaps remaining conv work; the final att batch is emitted directly after
the last conv evac; the last blocks' output pieces are deferred until after
the final block's broadcast DMAs (out-drain traffic otherwise delays the
small critical-path DMAs on the shared DMA semaphore lanes); pe_keep dummy
matmuls hold the PE clock gate up across tail dependency stalls.
"""

import sys
from contextlib import ExitStack

import numpy as np
import ml_dtypes

for _p in ("/opt/trn_rl_repo", "/root/.axon_site/_ro/trn_rl_repo"):
    if _p not in sys.path:
        sys.path.append(_p)

import concourse.bass as bass
import concourse.mybir as mybir
import concourse.tile as tile
from concourse.bass_utils import run_bass_kernel_spmd

F32 = mybir.dt.float32
BF16 = mybir.dt.bfloat16
AF = mybir.ActivationFunctionType

# Problem constants (hardcoded per contract).
B, CIN, COUT, H, W = 4, 128, 128, 256, 256
BR = 64
EPS_BR = 1e-3
EPS_ATT = 1e-5
WP = W + 2            # padded row stride in SBUF
NF = 130              # f rows per core (128 + 1 halo each side)
G = 4                 # f rows per conv group
NGRP = 33             # 32 full groups + 1 tail group of 2 rows
NQ = 9                # att quads: Q0..Q7 full (16 f rows), Q8 partial (2)
XROWS = 12            # f rows per x input tile (3 conv groups)
NXT = 11              # number of x tiles
# attention blocks: (first out f-row, width). Tail blocks narrow so the final
# serial chain (sigmoid/shuffle/mul/out) is short, with their gating conv
# groups (b6<-g27, b7<-g29, b8<-g31, b9<-g32) staggered so each block's
# chain overlaps remaining conv work.
BLOCKS = [(1, 16), (17, 16), (33, 16), (49, 16), (65, 16), (81, 16),
          (97, 14), (111, 8), (119, 8), (127, 2)]
NBLK = len(BLOCKS)
ORDER = list(range(33))
BW = 16               # max block width (tile sizing)
BK = 2 * BW + 2       # att_rt partition dim: bw+2 att rows + bw pos2 rows


def _grp_rows(g):
    """(start_f_row, n_rows) of conv group g."""
    return (G * g, 2 if g == NGRP - 1 else G)


def _band_off(b):
    """Column offset of block b's strip in the band matrix."""
    return sum(6 * bw for _, bw in BLOCKS[:b])


def emit_core(tc, outs, ins):
    """Emit the per-core program. outs/ins are dicts of DRAM APs."""
    nc = tc.nc
    out_d = outs["out"]
    xh_d, wf_d = ins["xh"], ins["wf"]
    chs_d, chb_d, av_d = ins["chs"], ins["chb"], ins["av"]
    band_d, pos2_d = ins["band"], ins["pos2"]

    ctx = ExitStack()
    with ctx:
        const = ctx.enter_context(tc.tile_pool(name="const", bufs=1))
        xp = ctx.enter_context(tc.tile_pool(name="xp", bufs=4))
        fbp = ctx.enter_context(tc.tile_pool(name="fbp", bufs=6))
        attq = ctx.enter_context(tc.tile_pool(name="attq", bufs=5))
        mapp = ctx.enter_context(tc.tile_pool(name="mapp", bufs=3))
        fps = ctx.enter_context(tc.tile_pool(name="fps", bufs=2, space="PSUM"))
        atps = ctx.enter_context(tc.tile_pool(name="atps", bufs=1, space="PSUM"))
        z2ps = ctx.enter_context(tc.tile_pool(name="z2ps", bufs=2, space="PSUM"))

        # Constants. The first x piece is issued by the caller loop before
        # these (sync queue); constants ride the scalar queue concurrently.
        wf_sb = const.tile([CIN, 9 * COUT], BF16)
        nc.scalar.dma_start(wf_sb[:], wf_d[:])
        chs_sb = const.tile([COUT, 1], F32)
        nc.scalar.dma_start(chs_sb[:], chs_d[:])
        chb_sb = const.tile([COUT, 1], F32)
        nc.scalar.dma_start(chb_sb[:], chb_d[:])
        av_sb = const.tile([CIN, 2], BF16)
        nc.scalar.dma_start(av_sb[:], av_d[:])
        band_sb = const.tile([BK, 6 * 128], BF16)
        nc.scalar.dma_start(band_sb[:], band_d[:])
        pos2_sb = const.tile([128, 2 * W], BF16)
        nc.scalar.dma_start(pos2_sb[:], pos2_d[:])
        # Persistent per-block att_rt tiles (~1 KB each); memset once zeroes
        # the x-pad columns (0 / 257 of each branch) for good, and the pos2
        # rows (constants) are staged for ALL blocks up front so no pos2 DMA
        # ever lands on the contended tail queues.
        att_rt = [const.tile([BK, 2 * WP], BF16, name=f"att_rt{i}")
                  for i in range(NBLK)]
        for i in range(NBLK):
            nc.gpsimd.memset(att_rt[i][:], 0.0)
        for b, (o0, bw) in enumerate(BLOCKS):
            rt = att_rt[b]
            dst = rt[bw + 2:2 * bw + 2, :].rearrange(
                "p (b w) -> p b w", b=2)[:, :, 1:1 + W]
            nc.sync.dma_start(dst, pos2_sb[o0 - 1:o0 - 1 + bw, :])
        # map4 staging tiles (partitions 0/32/64/96 hold map values); memset
        # once so stream_shuffle never reads uninitialized SBUF.
        map4 = [const.tile([128, BW * W], BF16, name=f"map4_{i}")
                for i in range(3)]
        for m in map4:
            nc.gpsimd.memset(m[:], 0.0)

        x_t = [None] * NXT
        FB = [None] * 9              # 16-f-row block -> SBUF f tile (bf16)
        att_sb = [None] * NQ         # quad -> SBUF att tile [128, 1024] bf16
        att_ps = [None] * NQ         # quad -> PSUM att tile
        pending_out = []             # deferred output pieces
        blocks_done = [False] * NBLK
        rows_done = [False] * NF     # f rows evac'd + repacked (out of order)
        map4_idx = 0
        keep_state = {"ps": None}

        def pe_keep(n):
            """Dummy PE matmuls that keep the HAM clock gate up across tail
            idle gaps (a briefly idle PE drops to half clock, slowing every
            matmul after it). Writes land in a retired conv PSUM bank."""
            if keep_state["ps"] is None:
                keep_state["ps"] = fps.tile([COUT, 512], F32, tag="big",
                                            name="keep_ps")
            kp = keep_state["ps"]
            for _ in range(n):
                nc.tensor.matmul(kp[0:64, :256], warm[:, 512:576],
                                 warm[:, 0:256], start=True, stop=True)

        def emit_repack_rows(lo_all, hi_all):
            """Repack newly evac'd att rows [lo_all, hi_all) into row-layout
            block tiles. Every block's first-touching batch contains its
            bottom halo row blo (which also stages its pos2 rows). All rows
            of one call lie within a single att quad; whole 4-row groups move
            as one multi-dim-AP DMA to cut dispatch count."""
            for b, (o0, bw) in enumerate(BLOCKS):
                blo, bhi = o0 - 1, o0 + bw + 1
                lo, hi = max(lo_all, blo), min(hi_all, bhi)
                if lo >= hi:
                    continue
                rt = att_rt[b]
                for br in range(2):
                    # src: att_sb[q] partitions 32i+br, cols j*W for att row
                    # r=16q+4i+j. Full groups batch into one [ni, 4, W] DMA.
                    # The two branches ride different engine queues.
                    eng = nc.scalar if br == 0 else nc.sync
                    r = lo
                    while r < hi:
                        q, i, j0 = r // 16, (r % 16) // 4, r % 4
                        if j0 == 0 and hi - r >= 4:
                            ni = (hi - r) // 4
                            src = att_sb[q][
                                32 * i + br:32 * (i + ni - 1) + br + 1:32,
                                :].rearrange("i (j w) -> i j w", j=4)
                            dst = rt[r - blo:r - blo + 4 * ni,
                                     br * WP + 1:br * WP + 1 + W]
                            eng.dma_start(dst, src, single_packet=True)
                            r += 4 * ni
                        else:
                            j1 = min(4, j0 + (hi - r))
                            nseg = j1 - j0
                            src = att_sb[q][32 * i + br:32 * i + br + 1,
                                            j0 * W:j1 * W]
                            eng.dma_start(
                                rt[r - blo:r - blo + nseg,
                                   br * WP + 1:br * WP + 1 + W], src,
                                single_packet=True)
                            r += nseg

        def emit_block(b):
            """Banded 3x3 attention conv + sigmoid + map broadcast + output
            multiplies for block b."""
            nonlocal map4_idx
            o0, bw = BLOCKS[b]
            if b >= NBLK - 2:
                # tail blocks start after all conv work: pad the PE queue so
                # the clock gate stays hot while their repack DMAs land
                pe_keep(10 if b == NBLK - 2 else 24)
            kk = 2 * bw + 2
            rt = att_rt[b]
            # both branches in one N=512 matmul per dx: out partition o<bw is
            # branch a (valid cols 0..W), partition bw+o is branch b (valid
            # cols W..2W); the off-branch quadrants are computed but unread.
            z2 = z2ps.tile([2 * BW, 2 * W], F32, tag="z2", name="z2")
            rhs2 = rt[0:kk, :].rearrange("p (b w) -> p b w", b=2)
            for dx in range(3):
                cix = _band_off(b) + dx * 2 * bw
                nc.tensor.matmul(
                    z2[:2 * bw, :], band_sb[0:kk, cix:cix + 2 * bw],
                    rhs2[:, :, dx:dx + W],
                    start=(dx == 0), stop=(dx == 2))
            mapS = mapp.tile([2 * BW, 2 * W], BF16, name="mapS")
            nc.scalar.activation(mapS[:2 * bw, :], z2[:2 * bw, :], AF.Sigmoid)
            # broadcast: rows of this block to channel partitions 0/32/64/96
            m4 = map4[map4_idx]
            map4_idx = (map4_idx + 1) % 3
            engs = ((nc.sync, nc.scalar) if b == NBLK - 1
                    else (nc.sync, nc.gpsimd))
            for br, parts, eng in ((0, (0, 32), engs[0]),
                                   (1, (64, 96), engs[1])):
                src = mapS[br * bw:(br + 1) * bw, br * W:(br + 1) * W]
                for p in parts:
                    eng.dma_start(m4[p:p + 1, :bw * W], src,
                                  single_packet=True)
            mrep = mapp.tile([128, BW * W], BF16, name="mrep", bufs=3)
            # shuffle is capped at 1x mode; bitcast bf16 pairs to f32 to
            # halve the streamed element count
            nc.vector.stream_shuffle(mrep[:, :bw * W].bitcast(F32),
                                     m4[:, :bw * W].bitcast(F32), [0] * 32)
            if b < NBLK - 2:
                # deferred pieces ride ahead of this block's mults; the tail
                # blocks' pieces instead flush after the LAST block's m4
                # dispatches, keeping the final chain's small DMAs ahead of
                # the out-drain on the DMA queues
                flush_pending_out()
            elif b == NBLK - 1:
                flush_pending_out()
            # multiply per overlapping 16-row f tile (in-place, bf16); first
            # piece goes out now via cast-DMA, the rest one block later. The
            # second-to-last block defers ALL pieces: its out otherwise lands
            # ahead of the last block's repack/m4 DMAs on the shared DMA sem
            # lanes, making the final chain transitively wait on its drain.
            first = b < NBLK - 3 or b == NBLK - 1
            for beta in range(o0 // 16, (o0 + bw - 1) // 16 + 1):
                lo, hi = max(o0, 16 * beta), min(o0 + bw, 16 * beta + 16)
                if lo >= hi:
                    continue
                dst = FB[beta][:, (lo - 16 * beta) * W:(hi - 16 * beta) * W]
                nc.vector.tensor_mul(
                    dst, dst, mrep[:, (lo - o0) * W:(hi - o0) * W])
                if first:
                    cut = lo + (hi - lo) // 2 if hi - lo > 2 else hi
                    nc.gpsimd.dma_start(
                        out_d[:, lo - 1:cut - 1, :],
                        dst[:, 0:(cut - lo) * W])
                    first = False
                else:
                    cut = lo
                if cut < hi:
                    pending_out.append((out_d[:, cut - 1:hi - 1, :],
                                        dst[:, (cut - lo) * W:]))
            blocks_done[b] = True

        def flush_pending_out():
            while pending_out:
                o, src = pending_out.pop(0)
                nc.gpsimd.dma_start(o, src)

        def emit_att_batch(gs):
            """Col-tiled attention matmuls for groups gs (ascending, within
            one quad per call batch may span quad boundary only at Q8), then
            evacs (alternating scalar/vector) and repack of the new rows."""
            for g in gs:
                q = g // 4
                if q == NQ - 1:
                    # tiny tail quad: borrow a z2-pool bank for its PSUM
                    att_ps[q] = z2ps.tile([2, _grp_rows(g)[1] * W], F32,
                                          tag="z2", name="att8_ps")
                elif att_ps[q] is None:
                    att_ps[q] = atps.tile([128, G * W], F32, name="att_ps")
            # column-half-major order: adjacent matmuls sit at different
            # tile_positions (PE column groups), so they overlap on the array
            for c in range(2):
                for g in gs:
                    q, i = g // 4, g % 4
                    f0, n = _grp_rows(g)
                    if c >= n * W // 512:
                        continue
                    beta = f0 // 16
                    off = (f0 - 16 * beta) * W
                    nc.tensor.matmul(
                        att_ps[q][32 * i:32 * i + 2, c * 512:(c + 1) * 512],
                        av_sb[:],
                        FB[beta][:, off + c * 512:off + (c + 1) * 512],
                        start=True, stop=True, tile_position=(0, 32 * i))
            if att_sb[gs[0] // 4] is None:
                att_sb[gs[0] // 4] = attq.tile([128, G * W], BF16,
                                               name="att_sb")
            if len(gs) >= 3:
                # full quad: one wide vector copy (its cost is per-partition
                # free-dim elements, so [128,1024] costs the same as [2,1024];
                # unwritten PSUM partitions copy garbage nobody reads)
                q = gs[0] // 4
                nc.vector.tensor_copy(att_sb[q][:], att_ps[q][:])
            else:
                for g in gs:
                    q, i = g // 4, g % 4
                    _, n = _grp_rows(g)
                    src = att_ps[q][32 * i:32 * i + 2, :n * W]
                    dst = att_sb[q][32 * i:32 * i + 2, :n * W]
                    nc.scalar.copy(dst, src)
            lo = _grp_rows(gs[0])[0]
            hi = sum(_grp_rows(gs[-1]))
            for r in range(lo, hi):
                rows_done[r] = True
            emit_repack_rows(lo, hi)

        def flush_ready():
            """Emit blocks whose att rows are all evac'd + repacked."""
            for b, (o0, bw) in enumerate(BLOCKS):
                if not blocks_done[b] and all(rows_done[o0 - 1:o0 + bw + 1]):
                    emit_block(b)

        # PE warmup: dummy matmuls on a zeroed scratch tile while the first
        # x piece is still in flight, so the HAM clock gate reaches 8/8
        # before real work starts (~3.4us of sustained activity needed).
        warm = const.tile([CIN, 576], BF16, name="warm")
        nc.vector.memset(warm[:], 0.0)
        wps = fps.tile([COUT, 512], F32, tag="big", name="warm_ps")
        for _ in range(8):
            nc.tensor.matmul(wps[0:64, :512], warm[:, 512:576],
                             warm[:, 0:512], start=True, stop=True)

        # att batches keyed by emission slot t (pre-conv): quads (lag 2) for
        # the bulk, singles near the end. Emitted before the conv so the PE
        # has ready work even when the conv's PSUM buffer awaits its evac;
        # flush_ready() right after, so gated block chains start before the
        # next conv group occupies the PE.
        PRE_BATCH = {5: [0, 1, 2, 3], 9: [4, 5, 6, 7], 13: [8, 9, 10, 11],
                     17: [12, 13, 14, 15], 21: [16, 17, 18, 19],
                     25: [20, 21, 22, 23], 28: [24, 25, 26, 27],
                     30: [28, 29], 31: [30], 32: [31]}
        for t, g in enumerate(ORDER):
            f0, n = _grp_rows(g)
            if t in PRE_BATCH:
                emit_att_batch(PRE_BATCH[t])
            k = g // 3
            if x_t[k] is None:
                nrows = min(XROWS + 2, 132 - XROWS * k)
                x_t[k] = xp.tile([CIN, nrows, WP], BF16, name="x_t")
                cuts = [0, 6, 10, nrows] if k == 0 else [0, nrows]
                for c0, c1 in zip(cuts[:-1], cuts[1:]):
                    nc.sync.dma_start(
                        x_t[k][:, c0:c1, :],
                        xh_d[:, XROWS * k + c0:XROWS * k + c1, :])
            xr0 = f0 - XROWS * k  # group's first f row within the x tile
            # conv: 9 taps x row-pair matmuls
            f_ps = fps.tile([COUT, G * W], F32, tag="big", name="f_ps")
            for tap in range(9):
                ky, kx = tap // 3, tap % 3
                lhsT = wf_sb[:, tap * COUT:(tap + 1) * COUT]
                for j in range(n // 2):
                    rhs = x_t[k][:, xr0 + 2 * j + ky:xr0 + 2 * j + ky + 2,
                                 kx:kx + W]
                    nc.tensor.matmul(
                        f_ps[:, j * 512:(j + 1) * 512], lhsT,
                        rhs, start=(tap == 0), stop=(tap == 8))
            # evac with fused per-channel scale+bias into the 16-row f tile
            beta = f0 // 16
            if FB[beta] is None:
                FB[beta] = fbp.tile([COUT, 16 * W], BF16, name="FB")
            off = (f0 - 16 * beta) * W
            if t == len(ORDER) - 1:
                # final group: evac in halves so its att matmul (which the
                # last block chains on) can start after the first half
                h = n * W // 2
                for c0 in (0, h):
                    nc.scalar.activation(
                        FB[beta][:, off + c0:off + c0 + h],
                        f_ps[:, c0:c0 + h],
                        AF.Identity, bias=chb_sb[:], scale=chs_sb[:])
            else:
                nc.scalar.activation(FB[beta][:, off:off + n * W],
                                     f_ps[:, :n * W],
                                     AF.Identity, bias=chb_sb[:],
                                     scale=chs_sb[:])
            if g == NGRP - 1:
                # final group: its att batch goes ahead of the tail blocks'
                # sigmoid/m4 work in the scalar queue, so the last block's
                # banded matmul unblocks as early as possible
                emit_att_batch([32])
            flush_ready()
        flush_pending_out()
        assert all(blocks_done)


# ---------------------------------------------------------------- host side --

def _position_grids():
    i = np.arange(H, dtype=np.float64)
    j = np.arange(W, dtype=np.float64)
    gh = np.abs(i - H // 2 + 0.5) / float(H // 2)
    gw = np.abs(j - W // 2 + 0.5) / float(W // 2)
    GH = np.broadcast_to(gh[:, None], (H, W))
    GW = np.broadcast_to(gw[None, :], (H, W))
    pr = np.sqrt(GH ** 2 + GW ** 2)
    k = 2.0 / (pr.max() - pr.min())
    pr = k * pr + (1.0 - pr.max() * k)
    return GH, GW, pr


def _conv3x3_zp(x, w):
    """x: [C, H, W], w: [O, C, 3, 3] -> [O, H, W] zero-padded conv."""
    C, H_, W_ = x.shape
    O = w.shape[0]
    xp = np.pad(x, ((0, 0), (1, 1), (1, 1)))
    out = np.zeros((O, H_, W_), np.float64)
    for ky in range(3):
        for kx in range(3):
            out += np.einsum("oc,chw->ohw", w[:, :, ky, kx],
                             xp[:, ky:ky + H_, kx:kx + W_])
    return out


def fold_inputs(inp):
    """Host-side constant folding. Returns (shared constants, per-half consts)."""
    gh, gw, pr = _position_grids()
    Wf = np.zeros((COUT, CIN, 3, 3), np.float64)
    bf = np.zeros(COUT, np.float64)
    A = np.zeros((CIN, 2), np.float64)
    pos2 = np.zeros((2, H, W), np.float64)
    scales = np.array([float(np.asarray(inp["scale1"])),
                       float(np.asarray(inp["scale2"]))])
    bandw = np.zeros((2, 3, 3), np.float64)
    for bi, br in enumerate("ab"):
        k1 = np.asarray(inp[f"bn_{br}_gamma"], np.float64) / np.sqrt(
            np.asarray(inp[f"bn_{br}_var"], np.float64) + EPS_BR)
        Wf[bi * BR:(bi + 1) * BR] = (
            np.asarray(inp[f"conv_{br}_w"], np.float64) * k1[:, None, None, None])
        bf[bi * BR:(bi + 1) * BR] = (
            (np.asarray(inp[f"conv_{br}_b"], np.float64)
             - np.asarray(inp[f"bn_{br}_mean"], np.float64)) * k1
            + np.asarray(inp[f"bn_{br}_beta"], np.float64))
        k2 = (float(np.asarray(inp[f"att_bn_{br}_gamma"])[0])
              / np.sqrt(float(np.asarray(inp[f"att_bn_{br}_var"])[0]) + EPS_ATT))
        wa = np.asarray(inp[f"att_{br}_w"], np.float64)[0, :, 0, 0]
        s = scales[bi]
        A[bi * BR:(bi + 1) * BR, bi] = (wa[:BR] * k2 / s) if s != 0.0 else 0.0
        pos1 = (k2 * (wa[BR] * gh + wa[BR + 1] * gw
                      + float(np.asarray(inp[f"att_{br}_b"])[0])
                      - float(np.asarray(inp[f"att_bn_{br}_mean"])[0]))
                + float(np.asarray(inp[f"att_bn_{br}_beta"])[0]))
        attn_w = np.asarray(inp[f"attn_{br}_w"], np.float64)
        pos2[bi] = _conv3x3_zp(np.stack([pos1, gh, gw, pr]), attn_w)[0]
        bandw[bi] = attn_w[0, 0]
    ch_scale = np.repeat(scales, BR)
    shared = {
        # wf DRAM layout: [cin, tap, cout]
        "wf": np.ascontiguousarray(
            Wf.transpose(1, 2, 3, 0).reshape(CIN, 9 * COUT)).astype(
                ml_dtypes.bfloat16),
        "chs": ch_scale.reshape(COUT, 1).astype(np.float32),
        "chb": (bf * ch_scale).reshape(COUT, 1).astype(np.float32),
        "av": A.astype(ml_dtypes.bfloat16),
    }
    halves = []
    for half in range(2):
        r0 = half * 128
        band = np.zeros((BK, 6 * 128), np.float64)
        for b, (o0, bw) in enumerate(BLOCKS):
            base = _band_off(b)
            for i in range(bw + 2):
                fr_in = o0 - 1 + i
                absr = r0 - 1 + fr_in
                if not (0 <= absr < H):
                    continue
                for o in range(bw):
                    dy = i - o
                    if 0 <= dy <= 2:
                        for dx in range(3):
                            for br in range(2):
                                band[i, base + (dx * 2 + br) * bw + o] = \
                                    bandw[br, dy, dx]
            for o in range(bw):
                # identity rows add pos2 (staged in att_rt) at dx=1
                for br in range(2):
                    band[bw + 2 + o, base + (2 + br) * bw + o] = 1.0
        p2 = np.zeros((128, 2, W), np.float64)
        for ro in range(128):
            p2[ro] = pos2[:, r0 + ro]
        halves.append({
            "band": np.ascontiguousarray(band).astype(ml_dtypes.bfloat16),
            "pos2": p2.reshape(128, 2 * W).astype(ml_dtypes.bfloat16),
        })
    return shared, halves


def make_in_maps(inp):
    shared, halves = fold_inputs(inp)
    x = np.asarray(inp["x"], np.float32)
    in_maps = []
    for core in range(8):
        b, half = core // 2, core % 2
        r0 = half * 128
        xpad = np.pad(x[b], ((0, 0), (2, 2), (1, 1)))
        xh = np.ascontiguousarray(xpad[:, r0:r0 + 132, :]).astype(
            ml_dtypes.bfloat16)
        in_maps.append({"xh": xh, **shared, **halves[half]})
    return in_maps


def _split_matmul_waits(nc):
    """This walrus build accepts only ONE sync wait command per engine
    instruction struct. Move extra waits onto sequencer NoOps inserted just
    before the instruction: the engine queue is processed in order, so the
    sequencer blocks on the NoOp's waits before dispatching it."""
    cnt = 0
    for fn in nc.m.functions:
        for bb in fn.blocks:
            insts = bb.instructions
            i = 0
            while i < len(insts):
                ins = insts[i]
                if (not isinstance(ins, mybir.InstNoOp) and ins.is_executable()
                        and ins.sync_info is not None):
                    w = list(ins.sync_info.on_wait)
                    if len(w) > 1:
                        ins.sync_info = mybir.SyncInfo(
                            on_wait=[w[0]],
                            on_update=list(ins.sync_info.on_update))
                        for sw in w[1:]:
                            cnt += 1
                            nop = mybir.InstNoOp(
                                name=f"I-mmwait-{cnt}", ins=[], outs=[])
                            nop.engine = ins.engine
                            nop.sync_info = mybir.SyncInfo(
                                on_wait=[sw], on_update=[])
                            insts.insert(i, nop)
                            i += 1
                i += 1
    return cnt


_PROGRAM = None


def _build_program():
    global _PROGRAM
    if _PROGRAM is not None:
        return _PROGRAM
    from concourse._compat import axon_active
    nc = bass.Bass("TRN2", target_bir_lowering=False,
                   debug=not axon_active(), enable_asserts=False,
                   num_devices=8)
    ins = {
        "xh": nc.dram_tensor("xh", [CIN, 132, WP], BF16,
                             kind="ExternalInput").ap(),
        "wf": nc.dram_tensor("wf", [CIN, 9 * COUT], BF16,
                             kind="ExternalInput").ap(),
        "chs": nc.dram_tensor("chs", [COUT, 1], F32, kind="ExternalInput").ap(),
        "chb": nc.dram_tensor("chb", [COUT, 1], F32, kind="ExternalInput").ap(),
        "av": nc.dram_tensor("av", [CIN, 2], BF16, kind="ExternalInput").ap(),
        "band": nc.dram_tensor("band", [BK, 6 * 128], BF16,
                               kind="ExternalInput").ap(),
        "pos2": nc.dram_tensor("pos2", [128, 2 * W], BF16,
                               kind="ExternalInput").ap(),
    }
    outs = {
        "out": nc.dram_tensor("out", [COUT, 128, W], F32,
                              kind="ExternalOutput").ap(),
    }
    with tile.TileContext(nc) as tc:
        emit_core(tc, outs, ins)
    _split_matmul_waits(nc)
    _PROGRAM = nc
    return nc


def run_cores(inp, trace=False, **kw):
    """Run the SPMD kernel; returns (full output, BassKernelResults)."""
    nc = _build_program()
    in_maps = make_in_maps(inp)
    res = run_bass_kernel_spmd(nc, in_maps, core_ids=list(range(8)),
                               trace=trace, **kw)
    out = np.zeros((B, COUT, H, W), np.float32)
    for core in range(8):
        b, half = core // 2, core % 2
        out[b, :, half * 128:half * 128 + 128] = res.results[core]["out"]
    return out, res


def kernel(**inputs):
    out, _ = run_cores(inputs)
    return out

